# revision 1
# baseline (speedup 1.0000x reference)
"""BinaryLinear Trainium2 kernel.

Computes out = x @ (alpha * sign(W)).T + bias where
alpha = mean(|W|, axis=1) (per-output-row scale), for
x [4, 2048, 4096] f32, W [4096, 4096] f32, bias [4096] f32.

Sharding: 2D tensor-parallel over 8 cores — 2-way over tokens x 4-way over
out_features. Each core gets x^T slice [4096, 4096] (k, t), W^T slice
[4096, 1024] (k, o), bias slice [1024], and produces out slice [4096, 1024].
(Host-side marshaling pre-transposes x and W so the contraction dim lands on
SBUF partitions; all arithmetic — sign, alpha, cast, matmul, scale+bias —
runs on device.)

Device math: S = sign(W) is exact in bf16, so a single bf16 matmul pass
(fp32 PSUM accumulation) carries only the bf16 rounding of x (~1e-3 rel).
alpha is exact fp32: |W| accumulated over k-chunks on the vector engine,
then one 128-deep fp32 ones-matmul per 512-wide o-chunk folds partitions;
alpha and bias are applied in fp32 on the vector engine.
"""

import numpy as np

import concourse.bass as bass
import concourse.mybir as mybir
import concourse.tile as tile
from concourse import bacc
from concourse import bass_isa
from concourse.bass_utils import run_bass_kernel_spmd

F32 = mybir.dt.float32
BF16 = mybir.dt.bfloat16

# Full problem shape (hardcoded; kernel.py must be self-contained).
B, S, D_IN, D_OUT = 4, 2048, 4096, 4096
T_FULL = B * S  # 8192 tokens
R_T, C_O = 2, 4  # token-dim shards x out-feature shards = 8 cores
N_CORES = R_T * C_O


def build_nc(K, T, O, tch=128, reps=1, ablate=()):
    """Build + compile the per-core Bass program.

    K: contraction dim (multiple of 128)
    T: tokens per core (multiple of tch; tch multiple of 128)
    O: out features per core (multiple of 512)
    reps: repeat the whole computation (timing only; reps=1 for grading)
    ablate: timing diagnostics; subset of {"loads", "mms", "stores", "veps"}

    Schedule: W streams in k-chunk groups (both o-chunks per group) while a
    wave of NW t-chunks holds open PSUM groups that accumulate each k-group
    as it lands (staggered entries matched to x-load arrival; missed k-groups
    caught up after the stream). Once W is fully resident the remaining
    t-chunks run as ordinary full-k PSUM groups.
    """
    P = 128
    KC = K // P          # k-chunks of 128
    OC = O // 512        # o-chunks of 512
    NT = T // tch        # t-chunks
    TS = tch // P        # t-subtiles per chunk
    WKG = 2              # k-chunks per W staging load
    NG = KC // WKG       # W k-groups
    NW = min(4, NT)      # wavefront t-chunks (NW*OC PSUM banks)
    ESTRIDE = 1          # wave entry stagger, in k-groups
    assert KC % WKG == 0 and TS == 1, "wave schedule assumes tch == 128"

    nc = bacc.Bacc("TRN2", target_bir_lowering=False, debug=False)

    # host-pretiled inputs: each staged DMA reads one fully contiguous block
    xT = nc.dram_tensor("xT", [NT, P, KC, tch], F32, kind="ExternalInput")
    wT = nc.dram_tensor("wT", [NG, OC, P, WKG, 512], F32, kind="ExternalInput")
    bias = nc.dram_tensor("bias", [1, O], F32, kind="ExternalInput")
    out = nc.dram_tensor("out", [T, O], F32, kind="ExternalOutput")

    xT_v = xT.ap()
    wT_v = wT.ap()
    out_v = out.ap().rearrange("(nt p) o -> p nt o", p=P)

    with tile.TileContext(nc) as tc:
        import contextlib

        with contextlib.ExitStack() as ctx:
            const = ctx.enter_context(tc.tile_pool(name="const", bufs=1))
            st_pool = ctx.enter_context(tc.tile_pool(name="st", bufs=1))
            wstage_pool = ctx.enter_context(tc.tile_pool(name="wstage", bufs=4))
            partial_pool = ctx.enter_context(tc.tile_pool(name="partial", bufs=3))
            xstage_pool = ctx.enter_context(tc.tile_pool(name="xstage", bufs=2))
            xbf_pool = ctx.enter_context(
                tc.tile_pool(name="xbf", bufs=NW + 2)
            )
            out_pool = ctx.enter_context(tc.tile_pool(name="out_sb", bufs=2))
            row_pool = ctx.enter_context(tc.tile_pool(name="rows", bufs=1))
            psum_wave = ctx.enter_context(
                tc.tile_pool(name="psum_wave", bufs=NW * OC, space="PSUM")
            )

            # persistent tensors
            ST = st_pool.tile([P, KC, O], BF16, tag="ST")  # sign(W)^T
            alpha_bc = const.tile([P, O], F32, tag="alpha_bc")
            bias_bc = const.tile([P, O], F32, tag="bias_bc")
            alpha_acc = const.tile([P, O], F32, tag="alpha_acc")

            def body(_=None):
                entry = {nt: min(nt * ESTRIDE, NG - 1) for nt in range(NW)}
                mm_count = {}
                wave_psum = {}
                wave_xbf = {}

                def load_x(nt):
                    xstage = xstage_pool.tile([P, KC, tch], F32, tag="xstage")
                    if "loads" not in ablate:
                        nc.sync.dma_start(xstage[:], xT_v[nt])
                    else:
                        nc.vector.memset(xstage[:, 0, :1], 0.0)
                    xbf = xbf_pool.tile(
                        [P, KC, tch], BF16, tag="xbf", name=f"xbf{nt}"
                    )
                    nc.scalar.copy(xbf[:], xstage[:])
                    return xbf

                def mm(nt, oc, kc, xbf):
                    key = (nt, oc)
                    n = mm_count.get(key, 0)
                    if n == 0:
                        wave_psum[key] = psum_wave.tile(
                            [P, 512], F32, tag="pw", name=f"pw_{nt}_{oc}"
                        )
                    if "mms" not in ablate:
                        nc.tensor.matmul(
                            wave_psum[key][:],
                            xbf[:, kc, :],
                            ST[:, kc, oc * 512 : (oc + 1) * 512],
                            start=(n == 0),
                            stop=(n == KC - 1),
                        )
                    mm_count[key] = n + 1

                def epilogue(nt, ocs):
                    out_sb = out_pool.tile([P, len(ocs) * 512], F32, tag="out_sb")
                    for i, oc in enumerate(ocs):
                        osl = slice(oc * 512, (oc + 1) * 512)
                        psl = slice(i * 512, (i + 1) * 512)
                        pt = wave_psum.pop((nt, oc))
                        if "veps" not in ablate:
                            nc.vector.tensor_mul(
                                out_sb[:, psl], pt[:], alpha_bc[:, osl]
                            )
                            nc.vector.tensor_add(
                                out_sb[:, psl], out_sb[:, psl], bias_bc[:, osl]
                            )
                    if "stores" not in ablate:
                        nc.sync.dma_start(
                            out_v[:, nt, ocs[0] * 512 : (ocs[-1] + 1) * 512],
                            out_sb[:],
                        )

                # ---- W stream + wavefront (entry-0 x loads lead the queue)
                for nt in range(NW):
                    if entry[nt] == 0:
                        wave_xbf[nt] = load_x(nt)
                nc.sync.dma_start(bias_bc[:], bias.ap().to_broadcast((P, O)))
                for oc in range(OC):
                    nc.vector.memset(
                        alpha_acc[:, oc * 512 : (oc + 1) * 512], 0.0
                    )
                for g in range(NG):
                    ksl = slice(g * WKG, (g + 1) * WKG)
                    for oc in range(OC):
                        osl = slice(oc * 512, (oc + 1) * 512)
                        wstage = wstage_pool.tile([P, WKG, 512], F32, tag="wstage")
                        if "loads" not in ablate:
                            nc.sync.dma_start(wstage[:], wT_v[g, oc])
                        else:
                            nc.vector.memset(wstage[:, 0, :1], 0.0)
                        nc.scalar.activation(
                            ST[:, ksl, osl],
                            wstage[:],
                            mybir.ActivationFunctionType.Sign,
                        )
                        partial = partial_pool.tile([P, 512], F32, tag="partial")
                        nc.vector.tensor_reduce(
                            partial[:],
                            wstage[:].rearrange("p a b -> p b a"),
                            axis=mybir.AxisListType.X,
                            op=mybir.AluOpType.add,
                            apply_absolute_value=True,
                        )
                        nc.vector.tensor_add(
                            alpha_acc[:, osl], alpha_acc[:, osl], partial[:]
                        )
                    for nt in range(NW):
                        if entry[nt] == g and nt not in wave_xbf:
                            wave_xbf[nt] = load_x(nt)
                        if entry[nt] <= g:
                            for kc in range(g * WKG, (g + 1) * WKG):
                                for oc in range(OC):
                                    mm(nt, oc, kc, wave_xbf[nt])

                # ---- alpha finalize (exact fp32, GPSIMD partition all-reduce)
                alpha_ar = row_pool.tile([P, O], F32, tag="alpha_ar")
                nc.gpsimd.partition_all_reduce(
                    alpha_ar[:], alpha_acc[:], channels=P,
                    reduce_op=bass_isa.ReduceOp.add,
                )
                nc.vector.tensor_scalar_mul(alpha_bc[:], alpha_ar[:], 1.0 / K)

                # ---- wave catch-up (missed leading k-groups) + epilogues
                for nt in range(NW):
                    for kc in range(0, entry[nt] * WKG):
                        for oc in range(OC):
                            mm(nt, oc, kc, wave_xbf[nt])
                    for oc in range(OC):
                        assert mm_count[(nt, oc)] == KC
                    epilogue(nt, list(range(OC)))
                    wave_xbf.pop(nt)
                mm_count.clear()

                # ---- steady state
                for nt in range(NW, NT):
                    xbf = load_x(nt)
                    for kc in range(KC):
                        for oc in range(OC):
                            mm(nt, oc, kc, xbf)
                    epilogue(nt, list(range(OC)))
                mm_count.clear()

            if reps == 1:
                body()
            else:
                with tc.For_i(0, reps, 1) as _i:
                    body()

    nc.compile()
    return nc


_NC_CACHE = {}


def _get_nc(key):
    if key not in _NC_CACHE:
        _NC_CACHE[key] = build_nc(*key)
    return _NC_CACHE[key]


def pretile_x(x_slice, tch=128):
    """[T, K] f32 -> [NT, 128, KC, tch] pretiled (pure permutation)."""
    T, K = x_slice.shape
    P = 128
    v = x_slice.reshape(T // tch, tch, K // P, P)
    return np.ascontiguousarray(v.transpose(0, 3, 2, 1))


def pretile_w(w_slice, wkg=2):
    """[O, K] f32 -> [NG, OC, 128, WKG, 512] pretiled (pure permutation)."""
    O, K = w_slice.shape
    P = 128
    u = w_slice.T.reshape(K // (wkg * P), wkg, P, O // 512, 512)
    return np.ascontiguousarray(u.transpose(0, 3, 2, 1, 4))


def make_in_maps(x2, w, b):
    T_c = T_FULL // R_T
    O_c = D_OUT // C_O
    xT_halves = [pretile_x(x2[i * T_c : (i + 1) * T_c, :]) for i in range(R_T)]
    in_maps = []
    for core in range(N_CORES):
        i, j = core // C_O, core % C_O
        in_maps.append(
            {
                "xT": xT_halves[i],
                "wT": pretile_w(w[j * O_c : (j + 1) * O_c, :]),
                "bias": np.ascontiguousarray(b[j * O_c : (j + 1) * O_c])[None, :],
            }
        )
    return in_maps


def kernel(x, weight_real, bias):
    assert x.shape == (B, S, D_IN) and weight_real.shape == (D_OUT, D_IN)
    x2 = np.ascontiguousarray(np.asarray(x, dtype=np.float32).reshape(T_FULL, D_IN))
    w = np.asarray(weight_real, dtype=np.float32)
    b = np.asarray(bias, dtype=np.float32)

    T_c = T_FULL // R_T   # 4096
    O_c = D_OUT // C_O    # 1024

    in_maps = make_in_maps(x2, w, b)
    nc = _get_nc((D_IN, T_c, O_c))
    res = run_bass_kernel_spmd(nc, in_maps, core_ids=list(range(N_CORES)))

    out_full = np.empty((T_FULL, D_OUT), dtype=np.float32)
    for core in range(N_CORES):
        i, j = core // C_O, core % C_O
        out_full[i * T_c : (i + 1) * T_c, j * O_c : (j + 1) * O_c] = res.results[
            core
        ]["out"]
    return out_full.reshape(B, S, D_OUT)

